# revision 23
# baseline (speedup 1.0000x reference)
"""Trainium2 Bass kernel for the CGC (multi-task MoE) layer.

Reference computation (all-dense MoE, T=2 tasks, E=6 experts, EC=4 per task):
    h1 = relu(x @ We1[e] + be1[e])            [B, E, H1]
    h2 = relu(h1 @ We2[e] + be2[e])           [B, E, H2]
    g  = relu(x @ Wg1[t] + bg1[t])            [B, T, G]
    gate = softmax(g @ Wgs[t])                [B, T, EC]
    out[t, b, :] = sum_j gate[b, t, j] * h2[b, IDX[t, j], :]

Sharding: data-parallel over batch across 8 NeuronCores (B=8192 -> 1024
rows/core), weights replicated, no collectives.  The host pre-transposes each
x shard to xT[D, BC] so every matmul's contraction dim sits on SBUF
partitions with no on-device transposes:

    L1:  psum[h, b] += We1[d, h].T-block @ xT[d, b]      (lhsT=We1, rhs=xT)
    L2:  psum[b, o] += h1T[h, b].T-block @ We2[h, o]     (lhsT=h1T, rhs=We2)

be2 is folded into the L2 PSUM accumulation with a K=1 ones-matmul.
relu+gate scaling is fused on ScalarE (gate > 0 so relu(g*x) = g*relu(x)).
"""

import numpy as np

import concourse.bass as bass
import concourse.mybir as mybir
import concourse.tile as tile
from concourse import bacc

# ---- problem shapes (hardcoded per contest contract) ----
B, D, H1, H2, G = 8192, 2048, 1024, 512, 512
T, SPEC, SH = 2, 2, 2
E = T * SPEC + SH          # 6
EC = SPEC + SH             # 4
NCORES = 8
BC = B // NCORES           # 1024 per-core batch
TASK_EXPERTS = [[0, 1, 4, 5], [2, 3, 4, 5]]   # IDX rows

F32 = mybir.dt.float32
F32R = mybir.dt.float32r
BF16 = mybir.dt.bfloat16
P = 128

Relu = mybir.ActivationFunctionType.Relu
Exp = mybir.ActivationFunctionType.Exp
AxX = mybir.AxisListType.X


def build_program(bc=BC, bch=None, reps=1, mm_mode="f32r"):
    """Build the SPMD single-core program.

    bc: per-core batch size; bch: L1/gate moving free-dim chunk; reps: body
    repetitions (for wall-clock differencing); mm_mode: f32 | f32r | bf16.
    """
    if bch is None:
        bch = min(512, bc)
    assert bc % P == 0 and bc % bch == 0
    KT, HT, GT = D // P, H1 // P, G // P
    NB, NBT = bc // bch, bc // P

    # matmul-operand dtype: walrus requires f32r matmul inputs to be produced
    # as f32r, so the whole producer chain (DRAM tensor -> DMA -> tile) is
    # declared f32r in that mode (same bytes as f32 host-side).
    io_dt = {"bf16": BF16, "f32r": F32R, "f32": F32}[mm_mode]

    def mm(ap):
        return ap

    nc = bacc.Bacc("TRN2", target_bir_lowering=False, debug=False,
                   num_devices=NCORES)

    xT = nc.dram_tensor("xT", [D, bc], io_dt, kind="ExternalInput")
    We1 = nc.dram_tensor("We1", [E, D, H1], io_dt, kind="ExternalInput")
    be1 = nc.dram_tensor("be1", [E, H1], F32, kind="ExternalInput")
    We2 = nc.dram_tensor("We2", [E, H1, H2], io_dt, kind="ExternalInput")
    be2 = nc.dram_tensor("be2", [E, H2], io_dt, kind="ExternalInput")
    Wg1 = nc.dram_tensor("Wg1", [T, D, G], io_dt, kind="ExternalInput")
    bg1 = nc.dram_tensor("bg1", [T, G], F32, kind="ExternalInput")
    Wgs = nc.dram_tensor("Wgs", [T, G, EC], F32, kind="ExternalInput")
    # K=1 all-ones lhsT used to broadcast be2 into the L2 PSUM accumulation;
    # an input tensor because only DMA can produce f32r-tagged data.
    ones_d = nc.dram_tensor("ones", [1, P], io_dt, kind="ExternalInput")
    out = nc.dram_tensor("out", [T, bc, H2], F32, kind="ExternalOutput")

    with tile.TileContext(nc) as tc:
        with (
            tc.tile_pool(name="persist", bufs=1) as pp,
            tc.tile_pool(name="w1", bufs=4) as w1p,
            tc.tile_pool(name="w2", bufs=9) as w2p,
            tc.tile_pool(name="h1", bufs=1) as h1p,
            tc.tile_pool(name="tmp", bufs=2) as tmpp,
            tc.tile_pool(name="small", bufs=2) as smp,
            tc.tile_pool(name="psA", bufs=4, space="PSUM") as psA,
            tc.tile_pool(name="psB", bufs=4, space="PSUM") as psB,
        ):
            for _rep in range(reps):
                # ---- resident x (transposed); per-dtile DMAs so the first
                # matmuls start after 1/16 of the load.  The first gate
                # weight pair is DMA'd BEFORE xt so PE can start as soon as
                # xt[d=0] lands. ----
                xt = pp.tile([P, KT * bc], io_dt, tag="xt")
                xtv = xt[:].rearrange("p (n b) -> p n b", b=bc)

                pairs = [(t, g) for t in range(T) for g in range(GT)]
                w1vs = {}

                def load_gate_w1(t, g):
                    w1 = w1p.tile([P, KT * P], io_dt, tag="w1",
                                  name=f"w1g_{t}_{g}")
                    w1v = w1[:].rearrange("p (n h) -> p n h", h=P)
                    nc.sync.dma_start(
                        w1v,
                        Wg1.ap()[t].rearrange("(n p) g -> p n g", p=P)
                        [:, :, g * P:(g + 1) * P])
                    w1vs[(t, g)] = w1v

                # first gate weight, then xt tiles with the other two first-
                # group weights interleaved, so PE starts at ~xt[d0] arrival
                load_gate_w1(*pairs[0])
                xTv = xT.ap().rearrange("(n p) b -> n p b", p=P)
                # remaining gate weights interleaved into the xt stream so
                # later chain groups never stall on queued weight DMAs
                w1_at = {0: pairs[1], 1: pairs[2], 4: pairs[3]}
                for d in range(KT):
                    nc.sync.dma_start(xtv[:, d, :], xTv[d])
                    if d in w1_at:
                        load_gate_w1(*w1_at[d])

                ones = pp.tile([1, P], io_dt, tag="ones")
                nc.sync.dma_start(ones[:], ones_d[:])

                # ---- gate phase ----
                # Both tasks' gate-hidden activations live in one h1-pool
                # slot [P, T, GT, bc].  Chains run 4-wide with the d-loop
                # OUTER so early matmuls track the xt tiles as they arrive
                # from HBM instead of stalling on the full 8MB load.
                bgts, wgsvs = [], []
                for t in range(T):
                    bgt = smp.tile([P, GT], F32, tag="bg")
                    nc.sync.dma_start(
                        bgt[:], bg1.ap()[t].rearrange("(n p) -> p n", p=P))
                    bgts.append(bgt)
                    wgst = smp.tile([P, GT * EC], F32, tag="wgs")
                    wgsv = wgst[:].rearrange("p (n e) -> p n e", e=EC)
                    nc.sync.dma_start(
                        wgsv, Wgs.ap()[t].rearrange("(n p) e -> p n e", p=P))
                    wgsvs.append(wgsv)

                gt_all = h1p.tile([P, T * GT * bc], F32, tag="h1T")
                gtv = gt_all[:].rearrange("p (t n b) -> p t n b", n=GT, b=bc)
                # group 1 is 3 pairs (6 concurrent chains, psA+psB) to keep
                # PE fed while xt streams in; then groups of 2 pairs
                groups = [pairs[0:3], pairs[3:5], pairs[5:7], pairs[7:8]]
                for grp in groups:
                    for (t, g) in grp:
                        if (t, g) not in w1vs:
                            load_gate_w1(t, g)
                    chains = [(t, g, cb) for (t, g) in grp
                              for cb in range(NB)]
                    pss = {}
                    for i, c in enumerate(chains):
                        pool = psA if i < 4 else psB
                        pss[c] = pool.tile(
                            [P, bch], F32,
                            tag="psA" if i < 4 else "psB",
                            name=f"psg_{c[0]}_{c[1]}_{c[2]}")
                    for d in range(KT):
                        for (t, g, cb) in chains:
                            nc.tensor.matmul(
                                pss[(t, g, cb)][:],
                                w1vs[(t, g)][:, d, :],
                                xtv[:, d, cb * bch:(cb + 1) * bch],
                                start=(d == 0), stop=(d == KT - 1))
                    for (t, g, cb) in chains:
                        nc.scalar.activation(
                            gtv[:, t, g, cb * bch:(cb + 1) * bch],
                            pss[(t, g, cb)][:], Relu,
                            bias=bgts[t][:, g:g + 1])

                gates = []
                for t in range(T):
                    gates_t = pp.tile([P, NBT * EC], F32, tag=f"gates{t}")
                    gatesv = gates_t[:].rearrange("p (n e) -> p n e", e=EC)
                    for bt in range(NBT):
                        psz = psB.tile([P, EC], F32, tag="psB")
                        for g in range(GT):
                            nc.tensor.matmul(
                                psz[:],
                                gtv[:, t, g, bt * P:(bt + 1) * P],
                                wgsvs[t][:, g, :],
                                start=(g == 0), stop=(g == GT - 1))
                        mx = smp.tile([P, 1], F32, tag="mx")
                        nc.vector.reduce_max(mx[:], psz[:], axis=AxX)
                        sh = smp.tile([P, EC], F32, tag="sh")
                        nc.vector.tensor_scalar_sub(sh[:], psz[:], mx[:])
                        ex = smp.tile([P, EC], F32, tag="ex")
                        ssum = smp.tile([P, 1], F32, tag="ss")
                        nc.scalar.activation(ex[:], sh[:], Exp,
                                             accum_out=ssum[:])
                        rec = smp.tile([P, 1], F32, tag="rc")
                        nc.vector.reciprocal(rec[:], ssum[:])
                        nc.vector.tensor_scalar_mul(gatesv[:, bt, :], ex[:],
                                                    rec[:])
                    gates.append(gatesv)

                # ---- output accumulators ----
                accs = []
                for t in range(T):
                    acc = pp.tile([P, NBT * H2], F32, tag=f"acc{t}")
                    accs.append(acc[:].rearrange("p (n o) -> p n o", o=H2))

                # ---- expert loop ----
                for e in range(E):
                    w2vs = []
                    We2e = We2.ap()[e].rearrange("(n p) o -> n p o", p=P)
                    for ht in range(HT):
                        w2t = w2p.tile([P, H2], io_dt, tag="w2")
                        nc.sync.dma_start(w2t[:], We2e[ht])
                        w2vs.append(w2t)
                    be2t = smp.tile([1, H2], io_dt, tag="be2")
                    nc.sync.dma_start(be2t[:], be2.ap()[e][None, :])
                    be1t = smp.tile([P, HT], F32, tag="be1")
                    nc.sync.dma_start(
                        be1t[:], be1.ap()[e].rearrange("(n p) -> p n", p=P))

                    # be2 broadcast to [P, H2] once per expert (one K=1
                    # ones-matmul + copy), DVE-added into each L2 psum below
                    psb2 = psB.tile([P, H2], F32, tag="psB")
                    nc.tensor.matmul(psb2[:], ones[:], be2t[:],
                                     start=True, stop=True)
                    be2b = smp.tile([P, H2], F32, tag="be2b")
                    nc.scalar.copy(be2b[:], psb2[:])

                    h1 = h1p.tile([P, HT * bc], io_dt, tag="h1T")
                    h1v = h1[:].rearrange("p (n b) -> p n b", b=bc)

                    # L1: h1T[h, b] = relu(sum_d We1[d, h]^T x[d, b] + be1)
                    # the NB b-chunk chains interleave per-d so consecutive
                    # matmuls share the same stationary weights and chain
                    # boundaries overlap
                    for ht in range(HT):
                        w1 = w1p.tile([P, KT * P], io_dt, tag="w1")
                        w1v = w1[:].rearrange("p (n h) -> p n h", h=P)
                        nc.sync.dma_start(
                            w1v,
                            We1.ap()[e].rearrange("(n p) h -> p n h", p=P)
                            [:, :, ht * P:(ht + 1) * P])
                        pss1 = [psA.tile([P, bch], F32, tag="psA",
                                         name=f"ps1_{e}_{ht}_{cb}")
                                for cb in range(NB)]
                        for d in range(KT):
                            for cb in range(NB):
                                nc.tensor.matmul(
                                    pss1[cb][:],
                                    mm(w1v[:, d, :]),
                                    mm(xtv[:, d, cb * bch:(cb + 1) * bch]),
                                    start=(d == 0), stop=(d == KT - 1))
                        for cb in range(NB):
                            nc.scalar.activation(
                                h1v[:, ht, cb * bch:(cb + 1) * bch],
                                pss1[cb][:], Relu, bias=be1t[:, ht:ht + 1])

                    # L2 + gated accumulation; btile pairs interleave so
                    # consecutive matmuls share stationary h1 slices' weights
                    # partner (same w2 moving operand) and chain boundaries
                    # overlap
                    for bt0 in range(0, NBT, 2):
                        bts = [bt0, bt0 + 1]
                        pss2 = [psB.tile([P, H2], F32, tag="psB",
                                         name=f"ps2_{e}_{bt}")
                                for bt in bts]
                        for ht in range(HT):
                            for i, bt in enumerate(bts):
                                nc.tensor.matmul(
                                    pss2[i][:],
                                    mm(h1v[:, ht, bt * P:(bt + 1) * P]),
                                    w2vs[ht][:],
                                    start=(ht == 0), stop=(ht == HT - 1))
                        for i, bt in enumerate(bts):
                            ps2 = pss2[i]
                            nc.vector.tensor_add(ps2[:], ps2[:], be2b[:])
                            for t in range(T):
                                if e not in TASK_EXPERTS[t]:
                                    continue
                                j = TASK_EXPERTS[t].index(e)
                                gate_ap = gates[t][:, bt, j:j + 1]
                                if e == TASK_EXPERTS[t][0]:
                                    nc.scalar.activation(
                                        accs[t][:, bt, :], ps2[:], Relu,
                                        scale=gate_ap)
                                else:
                                    tmp = tmpp.tile([P, H2], F32, tag="tmp")
                                    nc.scalar.activation(tmp[:], ps2[:],
                                                         Relu, scale=gate_ap)
                                    nc.vector.tensor_add(accs[t][:, bt, :],
                                                         accs[t][:, bt, :],
                                                         tmp[:])
                                if e == TASK_EXPERTS[t][-1]:
                                    nc.sync.dma_start(
                                        out.ap()[t].rearrange(
                                            "(n p) o -> p n o", p=P)
                                        [:, bt, :],
                                        accs[t][:, bt, :])

    nc.compile()
    return nc


# ---------------------------------------------------------------------------
# host-side SPMD execution (mirrors bass_utils.run_bass_kernel_spmd's axon
# path, but keeps the jitted callable so repeat calls don't recompile)
# ---------------------------------------------------------------------------
class SpmdRunner:
    def __init__(self, nc, n_cores):
        import jax
        from jax.sharding import Mesh, PartitionSpec
        from jax.experimental.shard_map import shard_map
        from concourse.bass2jax import (_bass_exec_p, install_neuronx_cc_hook,
                                        partition_id_tensor)
        install_neuronx_cc_hook()
        self.jax = jax
        self.nc = nc
        self.n_cores = n_cores
        partition_name = (nc.partition_id_tensor.name
                          if nc.partition_id_tensor else None)
        in_names, out_names, out_avals, zero_outs = [], [], [], []
        for alloc in nc.m.functions[0].allocations:
            if not isinstance(alloc, mybir.MemoryLocationSet):
                continue
            name = alloc.memorylocations[0].name
            if alloc.kind == "ExternalInput":
                if name != partition_name:
                    in_names.append(name)
            elif alloc.kind == "ExternalOutput":
                out_names.append(name)
                shape = tuple(alloc.tensor_shape)
                dtype = mybir.dt.np(alloc.dtype)
                out_avals.append(jax.core.ShapedArray(shape, dtype))
                zero_outs.append(np.zeros(shape, dtype))
        all_in_names = list(in_names) + list(out_names)
        if partition_name is not None:
            all_in_names.append(partition_name)

        def _body(*args):
            operands = list(args)
            if partition_name is not None:
                operands.append(partition_id_tensor())
            outs = _bass_exec_p.bind(
                *operands,
                out_avals=tuple(out_avals),
                in_names=tuple(all_in_names),
                out_names=tuple(out_names),
                lowering_input_output_aliases=(),
                sim_require_finite=True,
                sim_require_nnan=True,
                nc=nc,
            )
            return tuple(outs)

        devices = jax.devices()[:n_cores]
        assert len(devices) == n_cores
        self.mesh = Mesh(np.asarray(devices), ("core",))
        n_args = len(in_names) + len(out_names)
        self.fn = jax.jit(
            shard_map(_body, mesh=self.mesh,
                      in_specs=(PartitionSpec("core"),) * n_args,
                      out_specs=(PartitionSpec("core"),) * len(out_names),
                      check_rep=False),
            keep_unused=True,
        )
        self.in_names = in_names
        self.out_names = out_names
        self.out_avals = out_avals
        self.zero_outs = zero_outs
        self.PartitionSpec = PartitionSpec

    def put_inputs(self, in_maps):
        jax = self.jax
        concat_in = [
            np.concatenate([np.asarray(m[name]) for m in in_maps], axis=0)
            for name in self.in_names
        ]
        concat_zeros = [
            np.zeros((self.n_cores * z.shape[0], *z.shape[1:]), z.dtype)
            for z in self.zero_outs
        ]
        sh = jax.sharding.NamedSharding(self.mesh, self.PartitionSpec("core"))
        return [jax.device_put(a, sh) for a in concat_in + concat_zeros]

    def run(self, args):
        out = self.fn(*args)
        self.jax.block_until_ready(out)
        return out

    def results(self, out_arrs):
        return [
            {name: np.asarray(out_arrs[i]).reshape(
                self.n_cores, *self.out_avals[i].shape)[c]
             for i, name in enumerate(self.out_names)}
            for c in range(self.n_cores)
        ]


_CACHE = {}


def _to_io(a, mm_mode):
    a = np.asarray(a, np.float32)
    if mm_mode == "bf16":
        import ml_dtypes
        return np.ascontiguousarray(a.astype(ml_dtypes.bfloat16))
    return np.ascontiguousarray(a)


def make_in_maps(x, We1, be1, We2, be2, Wg1, bg1, Wgs, mm_mode):
    shared = {
        "We1": _to_io(We1, mm_mode),
        "be1": np.ascontiguousarray(np.asarray(be1, np.float32)),
        "We2": _to_io(We2, mm_mode),
        "be2": _to_io(be2, mm_mode),
        "Wg1": _to_io(Wg1, mm_mode),
        "bg1": np.ascontiguousarray(np.asarray(bg1, np.float32)),
        "Wgs": np.ascontiguousarray(np.asarray(Wgs, np.float32)),
        "ones": _to_io(np.ones((1, P), np.float32), mm_mode),
    }
    x = np.asarray(x, np.float32)
    in_maps = []
    for c in range(NCORES):
        xs = x[c * BC:(c + 1) * BC]
        in_maps.append({"xT": _to_io(xs.T, mm_mode), **shared})
    return in_maps


def get_runner(mm_mode="f32r", reps=1):
    key = (mm_mode, reps)
    if key not in _CACHE:
        nc = build_program(reps=reps, mm_mode=mm_mode)
        _CACHE[key] = SpmdRunner(nc, NCORES)
    return _CACHE[key]


MM_MODE = "f32r"


def kernel(x, We1, be1, We2, be2, Wg1, bg1, Wgs):
    runner = get_runner(MM_MODE)
    in_maps = make_in_maps(x, We1, be1, We2, be2, Wg1, bg1, Wgs, MM_MODE)
    args = runner.put_inputs(in_maps)
    res = runner.results(runner.run(args))
    out = np.concatenate([r["out"] for r in res], axis=1)  # [T, B, H2]
    return np.ascontiguousarray(out.astype(np.float32))
